# revision 18
# baseline (speedup 1.0000x reference)
"""LSTM cell (B=4096, I=H=1024, fp32) on 8 Trainium2 NeuronCores.

Strategy
--------
Sharding: 2-D -- batch split 4 ways x hidden split 2 ways (8 cores).
Per core: gates^T = Wcat^T_chunk @ xh^T computed in the transposed
[hidden, batch] layout so every tensor DMAs in its natural layout (all
transposes/packing happen on the host in numpy).

All matmul operands travel as float16 (values are unit-normal, so fp16's
range is ample and its 11-bit mantissa matches fp32r's effective
precision) -- this halves HBM traffic vs fp32 while keeping the PE at
1 cycle/row.  PSUM accumulates in fp32.  L2 relative error ~1.6e-3.

Schedule (PE-bound: ~109.5 us of matmul, 115.7 us CoreSim total, zero
PE gaps between the first and last matmul):
  Phase 1: the four gates of hidden-block 0 accumulate k-OUTER across
  all 8 PSUM banks, paced by the streaming x chunks (8 matmuls per
  256KB-fp16 x chunk vs ~1.1 us of DMA), so the PE starts ~2.5 us in
  and never waits on the x preload.
  Phase 2: the remaining 12 (gate, hidden-block) m-tiles run kc-outer /
  n-inner (each weight chunk enters the PE array once).  The epilogue
  is split: tanh(f*c_prev + i*c~) needs only gates f/i/c~, so it runs
  while the o-gate m-tile is still on the PE; after the o-gate's
  sigmoid only the final multiply and store remain.  The last m-tile
  runs as four 256-wide quarter sweeps so only one 256-wide
  sigmoid+mul+store chain (~1.4 us) trails the final matmul.
Weights stream on the gpsimd (Pool) DMA queue, x/c/bias/outputs on the
sync queue, so neither stream head-blocks the other.  The epilogue
(c_t/h_t elementwise) runs in fp16 on DVE (2x throughput) with tanh on
the scalar engine.
"""
import numpy as np
import concourse.bacc as bacc
import concourse.mybir as mybir
import concourse.tile as tile
from concourse.bass_utils import run_bass_kernel_spmd

B, I, H = 4096, 1024, 1024
BS, HS = 4, 2          # batch shards x hidden shards = 8 cores
BC = B // BS           # 1024 batch rows per core
GC = H // HS           # 512 hidden cols per gate per core
K = I + H              # 2048 contraction
M = 4 * GC             # 2048 gate columns per core
KT = K // 128          # 16 k-chunks
QT = GC // 128         # 4 hidden-row blocks per core
NT = BC // 512         # 2 batch halves

N_PASS = 1             # kept for test.py compat; ignored

f32 = mybir.dt.float32
f16 = mybir.dt.float16
AF = mybir.ActivationFunctionType


def build_nc(n_pass=N_PASS):
    nc = bacc.Bacc(None, target_bir_lowering=False)
    xh_t = nc.dram_tensor("xh_t", [K, BC], f16, kind="ExternalInput")
    wp1 = nc.dram_tensor("wp1", [KT, 128, 4, 128], f16, kind="ExternalInput")
    wp2 = nc.dram_tensor("wp2", [(QT - 1) * 4, 128, KT, 128], f16,
                         kind="ExternalInput")
    c_t = nc.dram_tensor("c_t", [GC, BC], f16, kind="ExternalInput")
    bias = nc.dram_tensor("bias", [M], f32, kind="ExternalInput")
    ht = nc.dram_tensor("ht_t", [GC, BC], f16, kind="ExternalOutput")

    with tile.TileContext(nc) as tc:
        with (
            tc.tile_pool(name="xpool", bufs=1) as xpool,
            tc.tile_pool(name="cpool", bufs=1) as cpool,
            tc.tile_pool(name="w1pool", bufs=1) as w1pool,
            tc.tile_pool(name="w2pool", bufs=3) as w2pool,
            tc.tile_pool(name="gpool", bufs=2) as gpool,
            tc.tile_pool(name="epool", bufs=3) as epool,
            tc.tile_pool(name="psum", bufs=1, space="PSUM") as psum,
        ):
            xh_r = xh_t.rearrange("(k p) b -> k p b", p=128)
            c_r = c_t.rearrange("(q p) b -> q p b", p=128)

            # sync queue, in order of first use: the 16 x chunks (phase-1
            # pacing), then c0 + bias (hm0 epilogue, ~28us in), then c1-c3
            # (needed at ~55/82/109us) so they yield HBM to phase-1 traffic
            # under real-hardware contention.
            x_sb = []
            for kc in range(KT):
                t = xpool.tile([128, BC], f16, tag=f"x{kc}", name="x")
                nc.sync.dma_start(t[:], xh_r[kc])
                x_sb.append(t)
            c_sb = []
            for q in range(QT):
                t = cpool.tile([128, BC], f16, tag=f"c{q}", name="c")
                c_sb.append(t)
            nc.sync.dma_start(c_sb[0][:], c_r[0])
            bias_sb = cpool.tile([128, 4 * QT], f32, tag="bias")
            nc.sync.dma_start(bias_sb[:], bias.rearrange("(c p) -> p c", p=128))
            for q in range(1, QT):
                nc.sync.dma_start(c_sb[q][:], c_r[q])

            def xs(kc, n):
                return x_sb[kc][:, n * 512:(n + 1) * 512]

            gtiles = {}

            def act_gate(g, hm, n, acc, chunk=512):
                gt = gpool.tile([128, 512], f16, tag=f"g{g}n{n}", name="gt")
                func = AF.Tanh if g == 2 else AF.Sigmoid
                mcol = g * QT + hm
                for s in range(512 // chunk):
                    cs = slice(s * chunk, (s + 1) * chunk)
                    nc.scalar.activation(gt[:, cs], acc[:, cs], func,
                                         bias=bias_sb[:, mcol:mcol + 1])
                gtiles[(g, hm, n)] = gt

            tctiles = {}

            def epilogue_pre(hm, n, chunk=256):
                # tanh(f*c_prev + i*c~): needs only gates 0-2, so it runs
                # while the o-gate m-tile is still on the PE
                f_t = gtiles.pop((0, hm, n))
                i_t = gtiles.pop((1, hm, n))
                ct_t = gtiles.pop((2, hm, n))
                tc_t = epool.tile([128, 512], f16, tag=f"tc{n}")
                for s in range(512 // chunk):
                    cs = slice(s * chunk, (s + 1) * chunk)
                    csl = c_sb[hm][:, n * 512 + s * chunk:
                                   n * 512 + (s + 1) * chunk]
                    t1 = epool.tile([128, chunk], f16, tag=f"t1_{chunk}")
                    nc.vector.tensor_mul(t1[:], f_t[:, cs], csl)
                    t2 = epool.tile([128, chunk], f16, tag=f"t2_{chunk}")
                    nc.vector.tensor_mul(t2[:], i_t[:, cs], ct_t[:, cs])
                    cn = epool.tile([128, chunk], f16, tag=f"cn_{chunk}")
                    nc.vector.tensor_add(cn[:], t1[:], t2[:])
                    nc.scalar.activation(tc_t[:, cs], cn[:], AF.Tanh)
                tctiles[(hm, n)] = tc_t

            def epilogue_post(hm, n, split_out=False):
                o_t = gtiles.pop((3, hm, n))
                tc_t = tctiles.pop((hm, n))
                ho = epool.tile([128, 512], f16, tag="ho")
                if split_out:
                    for s in range(2):
                        cs = slice(s * 256, (s + 1) * 256)
                        nc.vector.tensor_mul(ho[:, cs], o_t[:, cs],
                                             tc_t[:, cs])
                        nc.sync.dma_start(
                            ht[hm * 128:(hm + 1) * 128,
                               n * 512 + s * 256:n * 512 + (s + 1) * 256],
                            ho[:, cs])
                else:
                    nc.vector.tensor_mul(ho[:], o_t[:], tc_t[:])
                    nc.sync.dma_start(
                        ht[hm * 128:(hm + 1) * 128,
                           n * 512:(n + 1) * 512], ho[:])

            # ---- phase 1: hidden-block 0, k-outer across all 8 PSUM banks,
            # paced by the arriving x chunks ----
            accs = {(g, n): psum.tile([128, 512], f32, tag=f"a{g}{n}",
                                      name=f"a{g}{n}")
                    for g in range(4) for n in range(NT)}
            for kc in range(KT):
                w1 = w1pool.tile([128, 4, 128], f16, tag=f"w1_{kc}", name="w1")
                nc.gpsimd.dma_start(w1[:], wp1[kc])
                for g in range(4):
                    for n in range(NT):
                        nc.tensor.matmul(
                            accs[(g, n)][:], w1[:, g, :], xs(kc, n),
                            start=(kc == 0), stop=(kc == KT - 1))
            for g in range(4):
                for n in range(NT):
                    act_gate(g, 0, n, accs[(g, n)])
                if g == 2:
                    for n in range(NT):
                        epilogue_pre(0, n)
            for n in range(NT):
                epilogue_post(0, n)

            # ---- phase 2: remaining 12 m-tiles, kc-outer / n-inner so each
            # weight chunk is loaded into the PE array only once.  The very
            # last m-tile runs its two batch halves as separate sweeps so the
            # first half's epilogue and output DMA overlap the second sweep,
            # leaving only one short epilogue after the final matmul. ----
            for hm in range(1, QT):
                for g in range(4):
                    j = (hm - 1) * 4 + g
                    w2 = w2pool.tile([128, KT, 128], f16, tag="w2", name="w2")
                    nc.gpsimd.dma_start(w2[:], wp2[j])
                    last_tile = hm == QT - 1 and g == 3
                    if not last_tile:
                        accs2 = [psum.tile([128, 512], f32, tag=f"a{g}{n}",
                                           name=f"a{g}{n}")
                                 for n in range(NT)]
                        for kc in range(KT):
                            for n in range(NT):
                                nc.tensor.matmul(
                                    accs2[n][:], w2[:, kc, :], xs(kc, n),
                                    start=(kc == 0), stop=(kc == KT - 1))
                        for n in range(NT):
                            act_gate(g, hm, n, accs2[n])
                        if g == 2:
                            for n in range(NT):
                                epilogue_pre(hm, n)
                    else:
                        # last m-tile: four 256-wide quarter sweeps; each
                        # quarter's sigmoid + final mul + store overlaps the
                        # next quarter's matmuls, so only one 256-wide chain
                        # trails the very last matmul
                        for q in range(4):
                            n, half = divmod(q, 2)
                            acc = psum.tile([128, 256], f32,
                                            tag=f"a{g}{q % 2}", name="aL")
                            for kc in range(KT):
                                nc.tensor.matmul(
                                    acc[:], w2[:, kc, :],
                                    x_sb[kc][:, q * 256:(q + 1) * 256],
                                    start=(kc == 0), stop=(kc == KT - 1))
                            gt = gpool.tile([128, 256], f16, tag=f"g3q{q % 2}",
                                            name="gtq")
                            mcol = g * QT + hm
                            nc.scalar.activation(
                                gt[:], acc[:], AF.Sigmoid,
                                bias=bias_sb[:, mcol:mcol + 1])
                            tc_t = tctiles[(hm, n)]
                            hoq = epool.tile([128, 256], f16, tag="hoq")
                            nc.vector.tensor_mul(
                                hoq[:], gt[:],
                                tc_t[:, half * 256:(half + 1) * 256])
                            nc.sync.dma_start(
                                ht[hm * 128:(hm + 1) * 128,
                                   q * 256:(q + 1) * 256], hoq[:])
                        for n in range(NT):
                            tctiles.pop((hm, n), None)
                            gtiles.pop((3, hm, n), None)
                if hm < QT - 1:
                    for n in range(NT):
                        epilogue_post(hm, n)
    nc.compile()
    return nc


_NC_CACHE = {}


def _get_nc(n_pass=N_PASS):
    if 0 not in _NC_CACHE:
        _NC_CACHE[0] = build_nc()
    return _NC_CACHE[0]


def _make_in_maps(inputs, n_pass=N_PASS):
    f = lambda name: np.asarray(inputs[name], dtype=np.float32)
    xh = np.concatenate([f("x_t"), f("h_prev")], axis=1)            # [B, K]
    Wfull = np.concatenate([
        np.concatenate([f("W_f"), f("W_i"), f("W_c"), f("W_o")], axis=1),
        np.concatenate([f("U_f"), f("U_i"), f("U_c"), f("U_o")], axis=1),
    ], axis=0)                                                      # [K, 4H]
    bias_full = np.concatenate([f("b_f"), f("b_i"), f("b_c"), f("b_o")])
    c_prev = f("c_prev")

    in_maps = []
    for core in range(BS * HS):
        bi, hi = divmod(core, HS)
        cols = np.concatenate(
            [np.arange(g * H + hi * GC, g * H + (hi + 1) * GC)
             for g in range(4)])
        # [KT,128p,4g,QT,128mm] view of this core's weight block
        wc = Wfull[:, cols].reshape(KT, 128, 4, QT, 128)
        wp1 = np.ascontiguousarray(wc[:, :, :, 0, :], dtype=np.float16)
        wp2 = np.ascontiguousarray(
            wc[:, :, :, 1:, :].transpose(3, 2, 1, 0, 4), dtype=np.float16
        ).reshape((QT - 1) * 4, 128, KT, 128)
        im = {
            "xh_t": np.ascontiguousarray(
                xh[bi * BC:(bi + 1) * BC, :].T, dtype=np.float16),
            "wp1": wp1,
            "wp2": wp2,
            "c_t": np.ascontiguousarray(
                c_prev[bi * BC:(bi + 1) * BC, hi * GC:(hi + 1) * GC].T,
                dtype=np.float16),
            "bias": np.ascontiguousarray(bias_full[cols]),
        }
        in_maps.append(im)
    return in_maps


def _run(inputs, n_pass=N_PASS, **spmd_kwargs):
    nc = _get_nc()
    in_maps = _make_in_maps(inputs)
    res = run_bass_kernel_spmd(nc, in_maps, core_ids=list(range(BS * HS)),
                               **spmd_kwargs)
    h_t = np.empty((B, H), dtype=np.float32)
    for core in range(BS * HS):
        bi, hi = divmod(core, HS)
        h_t[bi * BC:(bi + 1) * BC, hi * GC:(hi + 1) * GC] = \
            res.results[core]["ht_t"].T.astype(np.float32)
    return h_t, res


def kernel(**inputs) -> np.ndarray:
    h_t, _ = _run(inputs)
    return h_t


# revision 43
# speedup vs baseline: 1.0006x; 1.0006x over previous
"""LSTM cell (B=4096, I=H=1024, fp32) on 8 Trainium2 NeuronCores.

Strategy
--------
Sharding: 2-D -- batch split 4 ways x hidden split 2 ways (8 cores).
Per core: gates^T = Wcat^T_chunk @ xh^T computed in the transposed
[hidden, batch] layout so every tensor DMAs in its natural layout (all
transposes/packing happen on the host in numpy).

All matmul operands travel as float16 (values are unit-normal, so fp16's
range is ample and its 11-bit mantissa matches fp32r's effective
precision) -- this halves HBM traffic vs fp32 while keeping the PE at
1 cycle/row.  PSUM accumulates in fp32.  L2 relative error ~1.6e-3.

Schedule (PE-bound: ~109.5 us of matmul, 115.7 us CoreSim total, zero
PE gaps between the first and last matmul):
  Phase 1: the four gates of hidden-block 0 accumulate k-OUTER across
  all 8 PSUM banks, paced by the streaming x chunks (8 matmuls per
  256KB-fp16 x chunk vs ~1.1 us of DMA), so the PE starts ~2.5 us in
  and never waits on the x preload.
  Phase 2: the remaining 12 (gate, hidden-block) m-tiles run kc-outer /
  n-inner (each weight chunk enters the PE array once).  The epilogue
  is split: tanh(f*c_prev + i*c~) needs only gates f/i/c~, so it runs
  while the o-gate m-tile is still on the PE; after the o-gate's
  sigmoid only the final multiply and store remain.  The last m-tile
  runs as four 256-wide quarter sweeps so only one 256-wide
  sigmoid+mul+store chain (~1.4 us) trails the final matmul.
Weights stream on the gpsimd (Pool) DMA queue, x/c/bias/outputs on the
sync queue, so neither stream head-blocks the other.  The epilogue
(c_t/h_t elementwise) runs in fp16 on DVE (2x throughput) with tanh on
the scalar engine.
"""
import numpy as np
import concourse.bacc as bacc
import concourse.mybir as mybir
import concourse.tile as tile
from concourse.bass_utils import run_bass_kernel_spmd

B, I, H = 4096, 1024, 1024
BS, HS = 4, 2          # batch shards x hidden shards = 8 cores
BC = B // BS           # 1024 batch rows per core
GC = H // HS           # 512 hidden cols per gate per core
K = I + H              # 2048 contraction
M = 4 * GC             # 2048 gate columns per core
KT = K // 128          # 16 k-chunks
QT = GC // 128         # 4 hidden-row blocks per core
NT = BC // 512         # 2 batch halves

N_PASS = 1             # kept for test.py compat; ignored

f32 = mybir.dt.float32
f16 = mybir.dt.float16
AF = mybir.ActivationFunctionType


def build_nc(n_pass=N_PASS):
    nc = bacc.Bacc(None, target_bir_lowering=False)
    xh_t = nc.dram_tensor("xh_t", [K, BC], f16, kind="ExternalInput")
    wp1 = nc.dram_tensor("wp1", [KT, 128, 4, 128], f16, kind="ExternalInput")
    wp2 = nc.dram_tensor("wp2", [(QT - 1) * 4, 128, KT, 128], f16,
                         kind="ExternalInput")
    c_t = nc.dram_tensor("c_t", [GC, BC], f16, kind="ExternalInput")
    bias = nc.dram_tensor("bias", [M], f32, kind="ExternalInput")
    ht = nc.dram_tensor("ht_t", [GC, BC], f16, kind="ExternalOutput")

    with tile.TileContext(nc) as tc:
        with (
            tc.tile_pool(name="xpool", bufs=1) as xpool,
            tc.tile_pool(name="cpool", bufs=1) as cpool,
            tc.tile_pool(name="w1pool", bufs=1) as w1pool,
            tc.tile_pool(name="w2pool", bufs=3) as w2pool,
            tc.tile_pool(name="gpool", bufs=2) as gpool,
            tc.tile_pool(name="epool", bufs=3) as epool,
            tc.tile_pool(name="psum", bufs=1, space="PSUM") as psum,
        ):
            xh_r = xh_t.rearrange("(k p) b -> k p b", p=128)
            c_r = c_t.rearrange("(q p) b -> q p b", p=128)

            # sync queue, in order of first use: the 16 x chunks (phase-1
            # pacing), then c0 + bias (hm0 epilogue, ~28us in), then c1-c3
            # (needed at ~55/82/109us) so they yield HBM to phase-1 traffic
            # under real-hardware contention.
            # the first matmuls' operands ride the sync (HWDGE) queue — its
            # sem path is ~500ns faster than gpsimd's SWDGE: k-chunk 0's
            # weights, then the two x half-chunks (n-outer kc0 matmuls need
            # only the first half to start)
            w1_first = w1pool.tile([128, 4, 128], f16, tag="w1_0", name="w1")
            nc.sync.dma_start(w1_first[:], wp1[0])
            x0h0 = xpool.tile([128, 512], f16, tag="x0h0", name="x0h0")
            nc.sync.dma_start(x0h0[:], xh_r[0][:, :512])
            x0h1 = xpool.tile([128, 512], f16, tag="x0h1", name="x0h1")
            nc.sync.dma_start(x0h1[:], xh_r[0][:, 512:])
            x_sb = [None]
            for kc in range(1, KT):
                t = xpool.tile([128, BC], f16, tag=f"x{kc}", name="x")
                nc.sync.dma_start(t[:], xh_r[kc])
                x_sb.append(t)
            c_sb = []
            for q in range(QT):
                t = cpool.tile([128, BC], f16, tag=f"c{q}", name="c")
                c_sb.append(t)
            nc.sync.dma_start(c_sb[0][:], c_r[0])
            bias_sb = cpool.tile([128, 4 * QT], f32, tag="bias")
            nc.sync.dma_start(bias_sb[:], bias.rearrange("(c p) -> p c", p=128))
            for q in range(1, QT):
                nc.sync.dma_start(c_sb[q][:], c_r[q])

            def xs(kc, n):
                if kc == 0:
                    return x0h0[:] if n == 0 else x0h1[:]
                return x_sb[kc][:, n * 512:(n + 1) * 512]

            gtiles = {}

            def act_gate(g, hm, n, acc, chunk=512):
                gt = gpool.tile([128, 512], f16, tag=f"g{g}n{n}", name="gt")
                func = AF.Tanh if g == 2 else AF.Sigmoid
                mcol = g * QT + hm
                for s in range(512 // chunk):
                    cs = slice(s * chunk, (s + 1) * chunk)
                    nc.scalar.activation(gt[:, cs], acc[:, cs], func,
                                         bias=bias_sb[:, mcol:mcol + 1])
                gtiles[(g, hm, n)] = gt

            tctiles = {}

            def epilogue_pre(hm, n, chunk=256):
                # tanh(f*c_prev + i*c~): needs only gates 0-2, so it runs
                # while the o-gate m-tile is still on the PE
                f_t = gtiles.pop((0, hm, n))
                i_t = gtiles.pop((1, hm, n))
                ct_t = gtiles.pop((2, hm, n))
                tc_t = epool.tile([128, 512], f16, tag=f"tc{n}")
                for s in range(512 // chunk):
                    cs = slice(s * chunk, (s + 1) * chunk)
                    csl = c_sb[hm][:, n * 512 + s * chunk:
                                   n * 512 + (s + 1) * chunk]
                    t1 = epool.tile([128, chunk], f16, tag=f"t1_{chunk}")
                    nc.vector.tensor_mul(t1[:], f_t[:, cs], csl)
                    t2 = epool.tile([128, chunk], f16, tag=f"t2_{chunk}")
                    nc.vector.tensor_mul(t2[:], i_t[:, cs], ct_t[:, cs])
                    cn = epool.tile([128, chunk], f16, tag=f"cn_{chunk}")
                    nc.vector.tensor_add(cn[:], t1[:], t2[:])
                    nc.scalar.activation(tc_t[:, cs], cn[:], AF.Tanh)
                tctiles[(hm, n)] = tc_t

            def epilogue_post(hm, n, split_out=False):
                o_t = gtiles.pop((3, hm, n))
                tc_t = tctiles.pop((hm, n))
                ho = epool.tile([128, 512], f16, tag="ho")
                if split_out:
                    for s in range(2):
                        cs = slice(s * 256, (s + 1) * 256)
                        nc.vector.tensor_mul(ho[:, cs], o_t[:, cs],
                                             tc_t[:, cs])
                        nc.sync.dma_start(
                            ht[hm * 128:(hm + 1) * 128,
                               n * 512 + s * 256:n * 512 + (s + 1) * 256],
                            ho[:, cs])
                else:
                    nc.vector.tensor_mul(ho[:], o_t[:], tc_t[:])
                    nc.sync.dma_start(
                        ht[hm * 128:(hm + 1) * 128,
                           n * 512:(n + 1) * 512], ho[:])

            # ---- phase 1: hidden-block 0, k-outer across all 8 PSUM banks,
            # paced by the arriving x chunks ----
            accs = {(g, n): psum.tile([128, 512], f32, tag=f"a{g}{n}",
                                      name=f"a{g}{n}")
                    for g in range(4) for n in range(NT)}
            for kc in range(KT):
                if kc == 0:
                    # n-OUTER so the first four matmuls touch only the first
                    # x half-chunk: the scheduler then places the second
                    # half's sem wait on matmul 5 and the PE starts as soon
                    # as 160KB (not 384KB) has landed
                    for n in range(NT):
                        for g in range(4):
                            nc.tensor.matmul(
                                accs[(g, n)][:], w1_first[:, g, :], xs(kc, n),
                                start=True, stop=False)
                    continue
                w1 = w1pool.tile([128, 4, 128], f16, tag=f"w1_{kc}", name="w1")
                nc.gpsimd.dma_start(w1[:], wp1[kc])
                for g in range(4):
                    for n in range(NT):
                        nc.tensor.matmul(
                            accs[(g, n)][:], w1[:, g, :], xs(kc, n),
                            start=False, stop=(kc == KT - 1))
            for g in range(4):
                for n in range(NT):
                    act_gate(g, 0, n, accs[(g, n)])
                if g == 2:
                    for n in range(NT):
                        epilogue_pre(0, n)
            for n in range(NT):
                epilogue_post(0, n)

            # ---- phase 2: remaining 12 m-tiles, kc-outer / n-inner so each
            # weight chunk is loaded into the PE array only once.  The very
            # last m-tile runs its two batch halves as separate sweeps so the
            # first half's epilogue and output DMA overlap the second sweep,
            # leaving only one short epilogue after the final matmul. ----
            for hm in range(1, QT):
                for g in range(4):
                    j = (hm - 1) * 4 + g
                    w2 = w2pool.tile([128, KT, 128], f16, tag="w2", name="w2")
                    nc.gpsimd.dma_start(w2[:], wp2[j])
                    last_tile = hm == QT - 1 and g == 3
                    if not last_tile:
                        accs2 = [psum.tile([128, 512], f32, tag=f"a{g}{n}",
                                           name=f"a{g}{n}")
                                 for n in range(NT)]
                        for kc in range(KT):
                            for n in range(NT):
                                nc.tensor.matmul(
                                    accs2[n][:], w2[:, kc, :], xs(kc, n),
                                    start=(kc == 0), stop=(kc == KT - 1))
                        for n in range(NT):
                            act_gate(g, hm, n, accs2[n])
                        if g == 2:
                            for n in range(NT):
                                epilogue_pre(hm, n)
                    else:
                        # last m-tile: four 256-wide quarter sweeps; each
                        # quarter's sigmoid + final mul + store overlaps the
                        # next quarter's matmuls, so only one 256-wide chain
                        # trails the very last matmul
                        for q in range(4):
                            n, half = divmod(q, 2)
                            acc = psum.tile([128, 256], f32,
                                            tag=f"a{g}{q % 2}", name="aL")
                            for kc in range(KT):
                                if kc == 0 and n == 0:
                                    xq = x0h0[:, half * 256:(half + 1) * 256]
                                elif kc == 0:
                                    xq = x0h1[:, half * 256:(half + 1) * 256]
                                else:
                                    xq = x_sb[kc][:, q * 256:(q + 1) * 256]
                                nc.tensor.matmul(
                                    acc[:], w2[:, kc, :], xq,
                                    start=(kc == 0), stop=(kc == KT - 1))
                            gt = gpool.tile([128, 256], f16, tag=f"g3q{q % 2}",
                                            name="gtq")
                            mcol = g * QT + hm
                            nc.scalar.activation(
                                gt[:], acc[:], AF.Sigmoid,
                                bias=bias_sb[:, mcol:mcol + 1])
                            tc_t = tctiles[(hm, n)]
                            hoq = epool.tile([128, 256], f16, tag="hoq")
                            nc.vector.tensor_mul(
                                hoq[:], gt[:],
                                tc_t[:, half * 256:(half + 1) * 256])
                            nc.sync.dma_start(
                                ht[hm * 128:(hm + 1) * 128,
                                   q * 256:(q + 1) * 256], hoq[:])
                        for n in range(NT):
                            tctiles.pop((hm, n), None)
                            gtiles.pop((3, hm, n), None)
                if hm < QT - 1:
                    for n in range(NT):
                        epilogue_post(hm, n)
    nc.compile()
    return nc


_NC_CACHE = {}


def _get_nc(n_pass=N_PASS):
    if 0 not in _NC_CACHE:
        _NC_CACHE[0] = build_nc()
    return _NC_CACHE[0]


def _make_in_maps(inputs, n_pass=N_PASS):
    f = lambda name: np.asarray(inputs[name], dtype=np.float32)
    xh = np.concatenate([f("x_t"), f("h_prev")], axis=1)            # [B, K]
    Wfull = np.concatenate([
        np.concatenate([f("W_f"), f("W_i"), f("W_c"), f("W_o")], axis=1),
        np.concatenate([f("U_f"), f("U_i"), f("U_c"), f("U_o")], axis=1),
    ], axis=0)                                                      # [K, 4H]
    bias_full = np.concatenate([f("b_f"), f("b_i"), f("b_c"), f("b_o")])
    c_prev = f("c_prev")

    in_maps = []
    for core in range(BS * HS):
        bi, hi = divmod(core, HS)
        cols = np.concatenate(
            [np.arange(g * H + hi * GC, g * H + (hi + 1) * GC)
             for g in range(4)])
        # [KT,128p,4g,QT,128mm] view of this core's weight block
        wc = Wfull[:, cols].reshape(KT, 128, 4, QT, 128)
        wp1 = np.ascontiguousarray(wc[:, :, :, 0, :], dtype=np.float16)
        wp2 = np.ascontiguousarray(
            wc[:, :, :, 1:, :].transpose(3, 2, 1, 0, 4), dtype=np.float16
        ).reshape((QT - 1) * 4, 128, KT, 128)
        xh_core = np.ascontiguousarray(
            xh[bi * BC:(bi + 1) * BC, :].T, dtype=np.float16)
        im = {
            "xh_t": xh_core,
            "wp1": wp1,
            "wp2": wp2,
            "c_t": np.ascontiguousarray(
                c_prev[bi * BC:(bi + 1) * BC, hi * GC:(hi + 1) * GC].T,
                dtype=np.float16),
            "bias": np.ascontiguousarray(bias_full[cols]),
        }
        in_maps.append(im)
    return in_maps


def _run(inputs, n_pass=N_PASS, **spmd_kwargs):
    nc = _get_nc()
    in_maps = _make_in_maps(inputs)
    res = run_bass_kernel_spmd(nc, in_maps, core_ids=list(range(BS * HS)),
                               **spmd_kwargs)
    h_t = np.empty((B, H), dtype=np.float32)
    for core in range(BS * HS):
        bi, hi = divmod(core, HS)
        h_t[bi * BC:(bi + 1) * BC, hi * GC:(hi + 1) * GC] = \
            res.results[core]["ht_t"].T.astype(np.float32)
    return h_t, res


def kernel(**inputs) -> np.ndarray:
    h_t, _ = _run(inputs)
    return h_t


# revision 50
# speedup vs baseline: 1.0014x; 1.0008x over previous
"""LSTM cell (B=4096, I=H=1024, fp32) on 8 Trainium2 NeuronCores.

Strategy
--------
Sharding: 2-D -- batch split 4 ways x hidden split 2 ways (8 cores).
Per core: gates^T = Wcat^T_chunk @ xh^T computed in the transposed
[hidden, batch] layout so every tensor DMAs in its natural layout (all
transposes/packing happen on the host in numpy).

All matmul operands travel as float16 (values are unit-normal, so fp16's
range is ample and its 11-bit mantissa matches fp32r's effective
precision) -- this halves HBM traffic vs fp32 while keeping the PE at
1 cycle/row.  PSUM accumulates in fp32.  L2 relative error ~1.6e-3.

Schedule (PE-bound: ~109.5 us of matmul, 115.7 us CoreSim total, zero
PE gaps between the first and last matmul):
  Phase 1: the four gates of hidden-block 0 accumulate k-OUTER across
  all 8 PSUM banks, paced by the streaming x chunks (8 matmuls per
  256KB-fp16 x chunk vs ~1.1 us of DMA), so the PE starts ~2.5 us in
  and never waits on the x preload.
  Phase 2: the remaining 12 (gate, hidden-block) m-tiles run kc-outer /
  n-inner (each weight chunk enters the PE array once).  The epilogue
  is split: tanh(f*c_prev + i*c~) needs only gates f/i/c~, so it runs
  while the o-gate m-tile is still on the PE; after the o-gate's
  sigmoid only the final multiply and store remain.  The last m-tile
  runs as four 256-wide quarter sweeps so only one 256-wide
  sigmoid+mul+store chain (~1.4 us) trails the final matmul.
Weights stream on the gpsimd (Pool) DMA queue, x/c/bias/outputs on the
sync queue, so neither stream head-blocks the other.  The epilogue
(c_t/h_t elementwise) runs in fp16 on DVE (2x throughput) with tanh on
the scalar engine.
"""
import numpy as np
import concourse.bacc as bacc
import concourse.mybir as mybir
import concourse.tile as tile
from concourse.bass_utils import run_bass_kernel_spmd

B, I, H = 4096, 1024, 1024
BS, HS = 4, 2          # batch shards x hidden shards = 8 cores
BC = B // BS           # 1024 batch rows per core
GC = H // HS           # 512 hidden cols per gate per core
K = I + H              # 2048 contraction
M = 4 * GC             # 2048 gate columns per core
KT = K // 128          # 16 k-chunks
QT = GC // 128         # 4 hidden-row blocks per core
NT = BC // 512         # 2 batch halves

N_PASS = 1             # kept for test.py compat; ignored

f32 = mybir.dt.float32
f16 = mybir.dt.float16
AF = mybir.ActivationFunctionType


def build_nc(n_pass=N_PASS):
    nc = bacc.Bacc(None, target_bir_lowering=False)
    # head1/head2 fuse the first matmuls' weights with the first two x
    # quarter-chunks so the PE's first dependency is ONE minimum-size DMA:
    # per-DMA queue-slice (500ns) + sem-prop (900ns) floors dominate, so
    # fewer/fused transfers ahead of the first matmul start the PE ~500ns
    # sooner.  head1 = [w1_0 gate0 | x0 cols 0:256], head2 = [w1_0 gates
    # 1-3 | x0 cols 256:512].
    head1 = nc.dram_tensor("head1", [128, 128 + 256], f16,
                           kind="ExternalInput")
    head2 = nc.dram_tensor("head2", [128, 384 + 256], f16,
                           kind="ExternalInput")
    xh_t = nc.dram_tensor("xh_t", [K, BC], f16, kind="ExternalInput")
    wp1 = nc.dram_tensor("wp1", [KT, 128, 4, 128], f16, kind="ExternalInput")
    wp2 = nc.dram_tensor("wp2", [(QT - 1) * 4, 128, KT, 128], f16,
                         kind="ExternalInput")
    c_t = nc.dram_tensor("c_t", [GC, BC], f16, kind="ExternalInput")
    bias = nc.dram_tensor("bias", [M], f32, kind="ExternalInput")
    ht = nc.dram_tensor("ht_t", [GC, BC], f16, kind="ExternalOutput")

    with tile.TileContext(nc) as tc:
        with (
            tc.tile_pool(name="xpool", bufs=1) as xpool,
            tc.tile_pool(name="cpool", bufs=1) as cpool,
            tc.tile_pool(name="w1pool", bufs=1) as w1pool,
            tc.tile_pool(name="w2pool", bufs=3) as w2pool,
            tc.tile_pool(name="gpool", bufs=2) as gpool,
            tc.tile_pool(name="epool", bufs=3) as epool,
            tc.tile_pool(name="psum", bufs=1, space="PSUM") as psum,
        ):
            xh_r = xh_t.rearrange("(k p) b -> k p b", p=128)
            c_r = c_t.rearrange("(q p) b -> q p b", p=128)

            # sync queue, in order of first use: the 16 x chunks (phase-1
            # pacing), then c0 + bias (hm0 epilogue, ~28us in), then c1-c3
            # (needed at ~55/82/109us) so they yield HBM to phase-1 traffic
            # under real-hardware contention.
            # the first matmuls' operands ride the sync (HWDGE) queue — its
            # sem path is ~500ns faster than gpsimd's SWDGE
            h1 = w1pool.tile([128, 384], f16, tag="h1", name="h1")
            nc.sync.dma_start(h1[:], head1[:])
            h2 = w1pool.tile([128, 640], f16, tag="h2", name="h2")
            nc.sync.dma_start(h2[:], head2[:])
            x0b = xpool.tile([128, 512], f16, tag="x0b", name="x0b")
            nc.sync.dma_start(x0b[:], xh_r[0][:, 512:])

            def w1_0g(g):
                return h1[:, :128] if g == 0 else h2[:, (g - 1) * 128:g * 128]

            def x0_frag(q):
                if q == 0:
                    return h1[:, 128:384]
                if q == 1:
                    return h2[:, 384:640]
                return x0b[:, (q - 2) * 256:(q - 1) * 256]
            x_sb = [None]
            for kc in range(1, KT):
                t = xpool.tile([128, BC], f16, tag=f"x{kc}", name="x")
                nc.sync.dma_start(t[:], xh_r[kc])
                x_sb.append(t)
            c_sb = []
            for q in range(QT):
                t = cpool.tile([128, BC], f16, tag=f"c{q}", name="c")
                c_sb.append(t)
            nc.sync.dma_start(c_sb[0][:], c_r[0])
            bias_sb = cpool.tile([128, 4 * QT], f32, tag="bias")
            nc.sync.dma_start(bias_sb[:], bias.rearrange("(c p) -> p c", p=128))
            for q in range(1, QT):
                nc.sync.dma_start(c_sb[q][:], c_r[q])

            def xs(kc, n):
                assert kc > 0
                return x_sb[kc][:, n * 512:(n + 1) * 512]

            gtiles = {}

            def act_gate(g, hm, n, acc, chunk=512):
                gt = gpool.tile([128, 512], f16, tag=f"g{g}n{n}", name="gt")
                func = AF.Tanh if g == 2 else AF.Sigmoid
                mcol = g * QT + hm
                for s in range(512 // chunk):
                    cs = slice(s * chunk, (s + 1) * chunk)
                    nc.scalar.activation(gt[:, cs], acc[:, cs], func,
                                         bias=bias_sb[:, mcol:mcol + 1])
                gtiles[(g, hm, n)] = gt

            tctiles = {}

            def epilogue_pre(hm, n, chunk=256):
                # tanh(f*c_prev + i*c~): needs only gates 0-2, so it runs
                # while the o-gate m-tile is still on the PE
                f_t = gtiles.pop((0, hm, n))
                i_t = gtiles.pop((1, hm, n))
                ct_t = gtiles.pop((2, hm, n))
                tc_t = epool.tile([128, 512], f16, tag=f"tc{n}")
                for s in range(512 // chunk):
                    cs = slice(s * chunk, (s + 1) * chunk)
                    csl = c_sb[hm][:, n * 512 + s * chunk:
                                   n * 512 + (s + 1) * chunk]
                    t1 = epool.tile([128, chunk], f16, tag=f"t1_{chunk}")
                    nc.vector.tensor_mul(t1[:], f_t[:, cs], csl)
                    t2 = epool.tile([128, chunk], f16, tag=f"t2_{chunk}")
                    nc.vector.tensor_mul(t2[:], i_t[:, cs], ct_t[:, cs])
                    cn = epool.tile([128, chunk], f16, tag=f"cn_{chunk}")
                    nc.vector.tensor_add(cn[:], t1[:], t2[:])
                    nc.scalar.activation(tc_t[:, cs], cn[:], AF.Tanh)
                tctiles[(hm, n)] = tc_t

            def epilogue_post(hm, n, split_out=False):
                o_t = gtiles.pop((3, hm, n))
                tc_t = tctiles.pop((hm, n))
                ho = epool.tile([128, 512], f16, tag="ho")
                if split_out:
                    for s in range(2):
                        cs = slice(s * 256, (s + 1) * 256)
                        nc.vector.tensor_mul(ho[:, cs], o_t[:, cs],
                                             tc_t[:, cs])
                        nc.sync.dma_start(
                            ht[hm * 128:(hm + 1) * 128,
                               n * 512 + s * 256:n * 512 + (s + 1) * 256],
                            ho[:, cs])
                else:
                    nc.vector.tensor_mul(ho[:], o_t[:], tc_t[:])
                    nc.sync.dma_start(
                        ht[hm * 128:(hm + 1) * 128,
                           n * 512:(n + 1) * 512], ho[:])

            # ---- phase 1: hidden-block 0, k-outer across all 8 PSUM banks,
            # paced by the arriving x chunks ----
            accs = {(g, n): psum.tile([128, 512], f32, tag=f"a{g}{n}",
                                      name=f"a{g}{n}")
                    for g in range(4) for n in range(NT)}
            for kc in range(KT):
                if kc == 0:
                    # quarter-outer: the first four matmuls need only head1
                    # (96KB).  start=True on each bank's first quarter marks
                    # the whole 2KB PSUM zero-region pending-zero, so the
                    # second quarter accumulates onto zeros with start=False.
                    for q in range(4):
                        n, half = divmod(q, 2)
                        for g in range(4):
                            acc = accs[(g, n)]
                            nc.tensor.matmul(
                                acc[:, half * 256:(half + 1) * 256],
                                w1_0g(g), x0_frag(q),
                                start=(half == 0), stop=False)
                    continue
                w1 = w1pool.tile([128, 4, 128], f16, tag=f"w1_{kc}", name="w1")
                nc.gpsimd.dma_start(w1[:], wp1[kc])
                for g in range(4):
                    for n in range(NT):
                        nc.tensor.matmul(
                            accs[(g, n)][:], w1[:, g, :], xs(kc, n),
                            start=False, stop=(kc == KT - 1))
            for g in range(4):
                for n in range(NT):
                    act_gate(g, 0, n, accs[(g, n)])
                if g == 2:
                    for n in range(NT):
                        epilogue_pre(0, n)
            for n in range(NT):
                epilogue_post(0, n)

            # ---- phase 2: remaining 12 m-tiles, kc-outer / n-inner so each
            # weight chunk is loaded into the PE array only once.  The very
            # last m-tile runs its two batch halves as separate sweeps so the
            # first half's epilogue and output DMA overlap the second sweep,
            # leaving only one short epilogue after the final matmul. ----
            for hm in range(1, QT):
                for g in range(4):
                    j = (hm - 1) * 4 + g
                    w2 = w2pool.tile([128, KT, 128], f16, tag="w2", name="w2")
                    nc.gpsimd.dma_start(w2[:], wp2[j])
                    last_tile = hm == QT - 1 and g == 3
                    if not last_tile:
                        accs2 = [psum.tile([128, 512], f32, tag=f"a{g}{n}",
                                           name=f"a{g}{n}")
                                 for n in range(NT)]
                        # kc0's n0 half is fragmented across head1/head2
                        for q in range(2):
                            nc.tensor.matmul(
                                accs2[0][:, q * 256:(q + 1) * 256],
                                w2[:, 0, :], x0_frag(q),
                                start=(q == 0), stop=False)
                        nc.tensor.matmul(accs2[1][:], w2[:, 0, :], x0b[:],
                                         start=True, stop=False)
                        for kc in range(1, KT):
                            for n in range(NT):
                                nc.tensor.matmul(
                                    accs2[n][:], w2[:, kc, :], xs(kc, n),
                                    start=False, stop=(kc == KT - 1))
                        for n in range(NT):
                            act_gate(g, hm, n, accs2[n])
                        if g == 2:
                            for n in range(NT):
                                epilogue_pre(hm, n)
                    else:
                        # last m-tile: four 256-wide quarter sweeps; each
                        # quarter's sigmoid + final mul + store overlaps the
                        # next quarter's matmuls, so only one 256-wide chain
                        # trails the very last matmul
                        for q in range(4):
                            n, half = divmod(q, 2)
                            acc = psum.tile([128, 256], f32,
                                            tag=f"a{g}{q % 2}", name="aL")
                            for kc in range(KT):
                                if kc == 0:
                                    xq = x0_frag(q)
                                else:
                                    xq = x_sb[kc][:, q * 256:(q + 1) * 256]
                                nc.tensor.matmul(
                                    acc[:], w2[:, kc, :], xq,
                                    start=(kc == 0), stop=(kc == KT - 1))
                            gt = gpool.tile([128, 256], f16, tag=f"g3q{q % 2}",
                                            name="gtq")
                            mcol = g * QT + hm
                            nc.scalar.activation(
                                gt[:], acc[:], AF.Sigmoid,
                                bias=bias_sb[:, mcol:mcol + 1])
                            tc_t = tctiles[(hm, n)]
                            hoq = epool.tile([128, 256], f16, tag="hoq")
                            nc.vector.tensor_mul(
                                hoq[:], gt[:],
                                tc_t[:, half * 256:(half + 1) * 256])
                            nc.sync.dma_start(
                                ht[hm * 128:(hm + 1) * 128,
                                   q * 256:(q + 1) * 256], hoq[:])
                        for n in range(NT):
                            tctiles.pop((hm, n), None)
                            gtiles.pop((3, hm, n), None)
                if hm < QT - 1:
                    for n in range(NT):
                        epilogue_post(hm, n)
    nc.compile()
    return nc


_NC_CACHE = {}


def _get_nc(n_pass=N_PASS):
    if 0 not in _NC_CACHE:
        _NC_CACHE[0] = build_nc()
    return _NC_CACHE[0]


def _make_in_maps(inputs, n_pass=N_PASS):
    f = lambda name: np.asarray(inputs[name], dtype=np.float32)
    xh = np.concatenate([f("x_t"), f("h_prev")], axis=1)            # [B, K]
    Wfull = np.concatenate([
        np.concatenate([f("W_f"), f("W_i"), f("W_c"), f("W_o")], axis=1),
        np.concatenate([f("U_f"), f("U_i"), f("U_c"), f("U_o")], axis=1),
    ], axis=0)                                                      # [K, 4H]
    bias_full = np.concatenate([f("b_f"), f("b_i"), f("b_c"), f("b_o")])
    c_prev = f("c_prev")

    in_maps = []
    for core in range(BS * HS):
        bi, hi = divmod(core, HS)
        cols = np.concatenate(
            [np.arange(g * H + hi * GC, g * H + (hi + 1) * GC)
             for g in range(4)])
        # [KT,128p,4g,QT,128mm] view of this core's weight block
        wc = Wfull[:, cols].reshape(KT, 128, 4, QT, 128)
        wp1 = np.ascontiguousarray(wc[:, :, :, 0, :], dtype=np.float16)
        wp2 = np.ascontiguousarray(
            wc[:, :, :, 1:, :].transpose(3, 2, 1, 0, 4), dtype=np.float16
        ).reshape((QT - 1) * 4, 128, KT, 128)
        xh_core = np.ascontiguousarray(
            xh[bi * BC:(bi + 1) * BC, :].T, dtype=np.float16)
        w1_0 = wp1[0].reshape(128, 512)
        im = {
            "head1": np.ascontiguousarray(
                np.concatenate([w1_0[:, :128], xh_core[0:128, 0:256]],
                               axis=1)),
            "head2": np.ascontiguousarray(
                np.concatenate([w1_0[:, 128:], xh_core[0:128, 256:512]],
                               axis=1)),
            "xh_t": xh_core,
            "wp1": wp1,
            "wp2": wp2,
            "c_t": np.ascontiguousarray(
                c_prev[bi * BC:(bi + 1) * BC, hi * GC:(hi + 1) * GC].T,
                dtype=np.float16),
            "bias": np.ascontiguousarray(bias_full[cols]),
        }
        in_maps.append(im)
    return in_maps


def _run(inputs, n_pass=N_PASS, **spmd_kwargs):
    nc = _get_nc()
    in_maps = _make_in_maps(inputs)
    res = run_bass_kernel_spmd(nc, in_maps, core_ids=list(range(BS * HS)),
                               **spmd_kwargs)
    h_t = np.empty((B, H), dtype=np.float32)
    for core in range(BS * HS):
        bi, hi = divmod(core, HS)
        h_t[bi * BC:(bi + 1) * BC, hi * GC:(hi + 1) * GC] = \
            res.results[core]["ht_t"].T.astype(np.float32)
    return h_t, res


def kernel(**inputs) -> np.ndarray:
    h_t, _ = _run(inputs)
    return h_t
